# revision 21
# baseline (speedup 1.0000x reference)
"""Trainium2 Bass kernel for nn_MixtureExpertsMlp (MoE soft routing).

Contract: kernel(**inputs) takes the FULL unsharded inputs
(x [4,4096,768], phi [4,1024,768], w1 [4,768,3072], b1 [4,3072],
w2 [4,3072,768], b2 [4,768]) and returns the FULL output [4,4096,768].

Sharding (expert+slot parallel over 8 NeuronCores): core c owns expert
e = c // 2 and slot half h = c % 2, i.e. SL = 512 of that expert's 1024
routing slots. Every core sees all tokens. Per core and per batch b:

  L^T[s, n]    = sum_d phi[s, d] x[b, n, d]        (slots on partitions)
  E^T          = exp(L^T)          (softmax max-subtraction skipped: the
                                    logits are ~N(0,1), well within range)
  ddenom[s]    = sum_n E^T[s, n]                    (via ACT accum_out)
  Et[n, s]     = E^T transposed per 128x128 block via PE matmul against a
                 bf16 identity (no ddenom dependency, so the transposes and
                 slot matmuls pipeline INSIDE phase 1, one tile behind exp)
  slotsU^T[d,s]= sum_n x[b, n, d] Et[n, s]          (unnormalized)
  slots^T[d,s] = slotsU^T[d, s] / ddenom[s]  (normalization folded into the
                 PSUM->SBUF copy: tensor_mul against a broadcast 1/ddenom
                 row, built via PE transpose + 1-partition ones matmul)
  h^T[h', s]   = gelu_tanh(sum_d w1[d, h'] slots^T[d, s] + b1[h'])
  y[s, od]     = sum_h h^T[h, s] w2[h, od]   (s on partitions directly:
                                    lhsT = h^T block, rhs = w2 rows)
  outp[n, :D]  = sum_s E^T[s, n] y[s, :]      (unnormalized combine)
  outp[n, D]   = sum_s E^T[s, n]              (ones column appended to y)

Host-side unshard: the combine softmax normalizer is global over all
E*S slots, so out = (sum_c num_c + sum_c gdl_c * b2[e(c)]) / sum_c gdl_c
where num_c = outp_c[..., :D] and gdl_c = outp_c[..., D]. This also
folds in b2 exactly (per-expert combine mass times b2[e]).

All matmul operands are bf16 (PSUM accumulation stays f32) EXCEPT the
logits and combine matmuls, which run in fp8 e4m3 DoubleRow mode (2x
MAC rate, K=256 per instruction, measured 216ns for a 512-free pair
vs 2x216ns in bf16). fp8 error analysis (numpy-simulated end to end):
quantizing e (the exp values, shifted by SHIFT so the max fits e4m3)
and y (pre-scaled by YSCALE via w2 so values clear the subnormal
floor) perturbs combine/dispatch weights incoherently ACROSS slots,
so the slot-averaged output error stays ~0.7-1%; quantizing x-content
(xn) or w1/w2 perturbs all slots coherently (~2.5-3.5%) and those
matmuls stay bf16. Predicted rel err 1.6e-2 vs the 2e-2 gate.
phi/w1/w2 are SBUF-resident across batches. Emission is
software-pipelined (transpose(v+1) before slots(v), mlp1(h+1) before
mlp2(h)) to keep the PE matmul pipe gap-free (gaps de-ramp the PE
clock 2.4 -> 1.2 GHz); eT is double-buffered so batch b+1's
logits/exp overlap batch b's combine drain.
"""

import numpy as np
import ml_dtypes
from contextlib import ExitStack

import concourse.bass as bass
import concourse.tile as tile
from concourse import mybir
from concourse.bass import ts
from concourse.bass_utils import run_bass_kernel_spmd

F32 = mybir.dt.float32
BF16 = mybir.dt.bfloat16
F8 = mybir.dt.float8e4
DR = mybir.MatmulPerfMode.DoubleRow
AF = mybir.ActivationFunctionType

N_CORES = 8
# softmax shift: exp(logit - SHIFT) keeps the stored fp8 e-values under the
# e4m3 max (240) for the worst logit (~7.2) while keeping the meaningful
# weight mass out of the subnormal range
SHIFT = 2.5
# y is tiny (~0.015 RMS); scale w2 by 16 on the host so the fp8 y_aug values
# sit in e4m3's normal range. The ones (gdl) column is also 16 so the
# num/den ratio on the host is unchanged.
YSCALE = 16.0


# --------------------------------------------------------------------------
# Post-pass: the walrus build in this container enforces the ISA cap of one
# sync-wait per instruction (two for EventSemaphore); Tile's final drain can
# carry more. Hoist excess waits onto fresh same-engine NOPs.
# --------------------------------------------------------------------------
def _split_excess_waits(nc):
    caps = {"InstEventSemaphore": 2}
    n_new = 0
    for f in nc.m.functions:
        for bb in f.blocks:
            i = 0
            insts = bb.instructions
            while i < len(insts):
                ins = insts[i]
                si = ins.sync_info
                cap = caps.get(type(ins).__name__, 1)
                if si is not None and len(si.on_wait) > cap:
                    waits = list(si.on_wait)
                    keep, hoist = waits[-cap:], waits[:-cap]
                    new_nops = []
                    for w in hoist:
                        nop = mybir.InstNoOp(
                            name=nc.get_next_instruction_name(),
                            engine=ins.engine,
                            ins=[],
                            outs=[],
                            sync_info=mybir.SyncInfo(on_wait=[w], on_update=[]),
                        )
                        nc.register_instruction(nop)
                        new_nops.append(nop)
                    ins.sync_info = mybir.SyncInfo(
                        on_wait=keep, on_update=list(si.on_update)
                    )
                    insts[i:i] = new_nops
                    i += len(new_nops)
                    n_new += len(new_nops)
                i += 1
    return n_new


def _bank_splits(off, width, bank=512):
    """Split [off, off+width) at PSUM-bank (512 f32) boundaries."""
    out, cur = [], off
    while cur < off + width:
        nxt = min((cur // bank + 1) * bank, off + width)
        out.append((cur, nxt - cur))
        cur = nxt
    return out


def _emit_moe_kernel(nc, B, N, D, SL, H, act_fn=AF.Gelu_apprx_tanh):
    assert N % 512 == 0 and D % 128 == 0 and SL % 128 == 0 and H % 128 == 0
    Dc, SLc, Hc = D // 128, SL // 128, H // 128
    NT, NV = N // 512, N // 128
    OD = D + 2  # ones column (combine denom) + pad for alignment

    xT = nc.dram_tensor("xT", [B, Dc, 128, N], F8, kind="ExternalInput").ap()
    xN = nc.dram_tensor("xN", [B, N, D], BF16, kind="ExternalInput").ap()
    phiT = nc.dram_tensor("phiT", [Dc, 128, SL], F8, kind="ExternalInput").ap()
    w1 = nc.dram_tensor("w1", [D, H], BF16, kind="ExternalInput").ap()
    w2 = nc.dram_tensor("w2", [H, D], BF16, kind="ExternalInput").ap()
    b1 = nc.dram_tensor("b1", [Hc, 128], F32, kind="ExternalInput").ap()
    identI = nc.dram_tensor("identI", [128, 128], F32, kind="ExternalInput").ap()
    identI8 = nc.dram_tensor("identI8", [128, 128], F8, kind="ExternalInput").ap()
    outp = nc.dram_tensor("outp", [B, N, OD], BF16, kind="ExternalOutput").ap()

    with tile.TileContext(nc) as tc, ExitStack() as ctx:
        pool = lambda name, bufs, space="SBUF": ctx.enter_context(
            tc.tile_pool(name=name, bufs=bufs, space=space)
        )
        singles = pool("singles", 1)
        eT_pool = pool("eT", 2)
        xt_pool = pool("xT", 2)
        xn_pool = pool("xN", 8)
        Dt_pool = pool("Dt", 8)
        slots_pool = pool("slots", 1)
        h_pool = pool("h", 3)
        y_pool = pool("y", 1)
        dd_pool = pool("dd", 2)
        norm_pool = pool("norm", 1)
        out_pool = pool("out", 6)

        # PSUM pools are opened per phase (stack allocator + overlap-deps
        # on release): each phase gets the accumulator banks it needs and
        # the combine phase reuses the freed banks for a deep pso rotation.
        ACC = Dc * 512
        assert ACC == SLc * 768

        # ---- SBUF residents (loaded once, reused across all batches).
        # w1/w2 DMAs (9.4MB) are deferred into batch 0's first tile so they
        # don't queue ahead of the x tiles the first matmuls need.
        phiT_s = singles.tile([128, Dc, SL], F8)
        nc.sync.dma_start(phiT_s[:], phiT.rearrange("k p m -> p k m"))
        w1_s = singles.tile([128, Dc, H], BF16)
        w2_s = singles.tile([128, Hc, D], BF16)
        b1_s = singles.tile([128, Hc], F32)
        nc.sync.dma_start(b1_s[:], b1.rearrange("o p -> p o"))
        ident = singles.tile([128, 128], F32)
        nc.sync.dma_start(ident[:], identI)
        ident_8 = singles.tile([128, 128], F8)
        nc.sync.dma_start(ident_8[:], identI8)
        # Broadcast-matmul weights: the PE rounds the K=1 contraction up to
        # 32 partitions and reads them all, so rows 1-31 must be REAL zeros
        # (garbage there would be accumulated into every output row).
        ones1 = singles.tile([32, 128], BF16)
        nc.vector.memset(ones1[:], 0.0)
        nc.vector.memset(ones1[0:1, :], 1.0)
        ebias = singles.tile([128, 1], F32)
        nc.vector.memset(ebias[:], -SHIFT)

        pref = {}  # batch -> prefetched (xt tile, [xn tiles]) for tile 0
        DEFER = 3  # combine v's per batch deferred into the next batch's
        #            phase 2->3 window (fills the slotsT-evacuation stall)
        pend_comb = [None]

        def emit_combine_v(bb, vv, pool, eT_t, y_t, act_only=False):
            ot = out_pool.tile([128, OD], BF16)
            for gi, (off, sz) in enumerate(((0, 448), (448, OD - 448))):
                pso = pool.tile([128, 512], F32, tag="pss", name="psO")
                for i in range(SLc // 2):
                    nc.tensor.matmul(
                        pso[:, :sz],
                        eT_t[:, 2 * i : 2 * i + 2, ts(vv, 128)],
                        y_t[:, 2 * i : 2 * i + 2, off : off + sz],
                        start=(i == 0),
                        stop=(i == SLc // 2 - 1),
                        perf_mode=DR,
                    )
                # alternate evacuation engines so neither serializes the
                # PSUM pool rotation; in the deferred window DVE is busy
                # with the slotsT muls, so use ACT for both pieces there
                if gi == 0 or act_only:
                    nc.scalar.copy(ot[:, off : off + sz], pso[:, :sz])
                else:
                    nc.vector.tensor_copy(ot[:, off : off + sz], pso[:, :sz])
            nc.sync.dma_start(outp[bb, ts(vv, 128), :], ot[:])

        for b in range(B):
            # ---- phase 1+2 fused: logits/exp, per-block PE transposes, and
            # ---- unnormalized slots accumulation in one dense PE stream ----
            eT = eT_pool.tile([128, SLc, N], F8)
            ddp = dd_pool.tile([128, SLc, NT], F32)
            ps12 = ctx12 = tc.tile_pool(name="ps12", bufs=2, space="PSUM")
            ps12 = ctx12.__enter__()
            psacc_ctx = tc.tile_pool(name="psacc", bufs=1, space="PSUM")
            ps_acc = psacc_ctx.__enter__()
            ps_small = ps12
            accS = ps_acc.tile([128, ACC], F32, tag="acc", name="accS")
            dts, xns = {}, {}

            def emit_T(v):
                psDt = ps_small.tile([128, 512], F32, tag="pss", name="psD")
                for s in range(SLc):
                    nc.tensor.matmul(
                        psDt[:, ts(s, 128)],
                        eT[:, s, ts(v, 128)],
                        ident_8[:],
                        start=True,
                        stop=True,
                    )
                Dt = Dt_pool.tile([128, SL], BF16)
                nc.vector.tensor_copy(Dt[:], psDt[:, :SL])
                dts[v] = Dt

            def emit_S(v):
                Dt, xn = dts.pop(v), xns.pop(v)
                for d in range(Dc):
                    nc.tensor.matmul(
                        accS[:, d * 512 : d * 512 + SL],
                        xn[:, ts(d, 128)],
                        Dt[:],
                        start=(v == 0),
                        stop=(v == NV - 1),
                    )

            # 1/ddenom broadcast-row chain, interleaved into the final tile's
            # T/S pairs so the PE never waits on the DVE/DMA latency:
            #   reduce+recip (DVE, after last exp) -> PE transpose
            #   [128,SLc]->[SLc,128] -> bf16 copy -> flatten DMAs ->
            #   1-partition ones matmul -> broadcast row in SBUF
            rdd = dd_pool.tile([128, SLc], F32, tag="rdd", name="rdd")
            pstS = norm_pool.tile([SLc, 128], BF16, tag="pstS", name="pstS")
            rddF = norm_pool.tile([1, SL], BF16, tag="rddF", name="rddF")
            rddB = norm_pool.tile([128, SL], F32, tag="rddB", name="rddB")

            def norm_chain_a():
                pst = ps_small.tile([128, 512], F32, tag="pss", name="psT")
                nc.tensor.transpose(pst[0:SLc, 0:128], rdd[:], ident[:])
                nc.vector.tensor_copy(pstS[:], pst[0:SLc, 0:128])
                # one DMA per source partition: a single rearranged
                # "p k -> (p k)" DMA under-reports its partition extent to
                # the Tile dependency tracker and races the copy above
                for k in range(SLc):
                    nc.sync.dma_start(rddF[0:1, ts(k, 128)], pstS[k : k + 1, :])

            def norm_chain_b():
                psB = ps_small.tile([128, 512], F32, tag="pss", name="psB")
                nc.tensor.matmul(
                    psB[:, :SL], ones1[0:1, :], rddF[:], start=True, stop=True
                )
                nc.vector.tensor_copy(rddB[:], psB[:, :SL])

            s_queue = []

            def emit_TS_for_tile(t, hooks=()):
                for i, v in enumerate(range(4 * t, 4 * t + 4)):
                    emit_T(v)
                    if s_queue:
                        emit_S(s_queue.pop(0))
                    s_queue.append(v)
                    for hook_i, hook in hooks:
                        if hook_i == i:
                            hook()

            for t in range(NT):
                if t == 0 and b in pref:
                    xt, pxns = pref.pop(b)
                else:
                    pxns = None
                    xt = xt_pool.tile([128, Dc, 512], F8)
                    nc.sync.dma_start(
                        xt[:], xT[b, :, :, ts(t, 512)].rearrange("k p n -> p k n")
                    )
                for s in range(SLc):
                    ps = ps_small.tile([128, 512], F32, tag="pss", name="psL")
                    for j in range(Dc // 2):
                        nc.tensor.matmul(
                            ps[:],
                            phiT_s[:, 2 * j : 2 * j + 2, ts(s, 128)],
                            xt[:, 2 * j : 2 * j + 2, :],
                            start=(j == 0),
                            stop=(j == Dc // 2 - 1),
                            perf_mode=DR,
                        )
                    nc.scalar.activation(
                        eT[:, s, ts(t, 512)],
                        ps[:],
                        AF.Exp,
                        bias=ebias[:],
                        accum_out=ddp[:, s, t : t + 1],
                    )
                for v in range(4 * t, 4 * t + 4):
                    if pxns is not None:
                        xns[v] = pxns[v - 4 * t]
                        continue
                    xn = xn_pool.tile([128, D], BF16)
                    nc.sync.dma_start(xn[:], xN[b, ts(v, 128), :])
                    xns[v] = xn
                if b == 0:
                    # stream the 9.4MB of resident weights in per-tile chunks
                    # so they never queue ahead of the x tiles phase 1 needs
                    hchunk = H // NT
                    nc.sync.dma_start(
                        w1_s[:, :, ts(t, hchunk)],
                        w1[:, ts(t, hchunk)].rearrange("(k p) m -> p k m", p=128),
                    )
                    kchunk = Hc // NT
                    nc.sync.dma_start(
                        w2_s[:, ts(t, kchunk), :],
                        w2[ts(t, kchunk * 128), :].rearrange(
                            "(k p) m -> p k m", p=128
                        ),
                    )
                if t == NT - 1:
                    nc.vector.reduce_sum(
                        rdd[:], ddp[:], axis=mybir.AxisListType.X
                    )
                    nc.vector.reciprocal(rdd[:], rdd[:])
                if t >= 1:
                    emit_TS_for_tile(t - 1)
            emit_TS_for_tile(NT - 1, hooks=((0, norm_chain_a), (3, norm_chain_b)))
            while s_queue:
                emit_S(s_queue.pop(0))

            # slots^T = slotsU^T * (1/ddenom), fused into the PSUM evacuation
            slotsT = slots_pool.tile([128, Dc, SL], BF16)
            for d in range(Dc):
                nc.vector.tensor_mul(
                    slotsT[:, d, :], accS[:, d * 512 : d * 512 + SL], rddB[:]
                )
            psacc_ctx.__exit__(None, None, None)
            ctx12.__exit__(None, None, None)
            ps3_ctx = tc.tile_pool(name="ps3", bufs=2, space="PSUM")
            ps_small = ps3_ctx.__enter__()
            psaccy_ctx = tc.tile_pool(name="psaccy", bufs=1, space="PSUM")
            ps_acc = psaccy_ctx.__enter__()

            # ---- phase 3: expert MLP; y accumulated with s on partitions ----
            accY = ps_acc.tile([128, ACC], F32, tag="acc", name="accY")
            # y_aug allocated up front so its gdl/pad memsets and the
            # per-s evacuation copies (interleaved into the final emit_h2
            # below) never gate the first combine matmul
            y_aug = y_pool.tile([128, SLc, OD], F8)
            nc.vector.memset(y_aug[:, :, D : D + 1], YSCALE)
            nc.vector.memset(y_aug[:, :, D + 1 : D + 2], 0.0)

            def emit_h1(h):
                psh = ps_small.tile([128, 512], F32, tag="pss", name="psH")
                for d in range(Dc):
                    nc.tensor.matmul(
                        psh[:, :SL],
                        w1_s[:, d, ts(h, 128)],
                        slotsT[:, d, :],
                        start=(d == 0),
                        stop=(d == Dc - 1),
                    )
                ht = h_pool.tile([128, SL], BF16)
                nc.scalar.activation(
                    ht[:], psh[:, :SL], act_fn, bias=b1_s[:, h : h + 1]
                )
                return ht

            def emit_h2(h, ht, final=False):
                for s in range(SLc):
                    for off, sz in _bank_splits(s * D, D):
                        # start=True clears the has_written bits of the WHOLE
                        # 2KB PSUM bank: only the bank-leading piece may carry
                        # it. A same-bank follower piece relies on that clear:
                        # its first start=False matmul overwrites (bit clear),
                        # later ones accumulate (bit set).
                        bank_first = off % 512 == 0
                        nc.tensor.matmul(
                            accY[:, off : off + sz],
                            ht[:, ts(s, 128)],
                            w2_s[:, h, off - s * D : off - s * D + sz],
                            start=(h == 0 and bank_first),
                            stop=(h == Hc - 1),
                            skip_group_check=not bank_first,
                        )
                    if final:
                        # s's accY region is complete: evacuate to y_aug now
                        # so the copies overlap the remaining s-blocks' matmuls
                        srcY = accY[:, s * D : (s + 1) * D]
                        h1 = D // 2
                        nc.vector.tensor_copy(y_aug[:, s, :h1], srcY[:, :h1])
                        nc.scalar.copy(y_aug[:, s, h1:D], srcY[:, h1:])

            pend_h = None
            for h in range(Hc):
                cur = emit_h1(h)
                if pend_h is not None:
                    emit_h2(h - 1, pend_h)
                pend_h = cur
            emit_h2(Hc - 1, pend_h, final=True)

            psaccy_ctx.__exit__(None, None, None)
            ps3_ctx.__exit__(None, None, None)
            # prefetch batch b+1's first x tiles ahead of the 32 output DMAs
            # so the next batch's logits stream never waits on the DMA queues
            if b + 1 < B:
                pxt = xt_pool.tile([128, Dc, 512], F8)
                nc.sync.dma_start(
                    pxt[:], xT[b + 1, :, :, ts(0, 512)].rearrange("k p n -> p k n")
                )
                pxn_list = []
                for v in range(4):
                    xn = xn_pool.tile([128, D], BF16)
                    nc.sync.dma_start(xn[:], xN[b + 1, ts(v, 128), :])
                    pxn_list.append(xn)
                pref[b + 1] = (pxt, pxn_list)
            ps4_ctx = tc.tile_pool(name="ps4", bufs=6, space="PSUM")
            ps_small = ps4_ctx.__enter__()
            # ---- phase 4: combine partials + local denominator ----
            # piece split 448/322 balances the ACT/DVE evacuation against the
            # halved (DoubleRow) PE time per v (~1540 cycles)
            for v in range(NV):
                ot = out_pool.tile([128, OD], BF16)
                for gi, (off, sz) in enumerate(((0, 448), (448, OD - 448))):
                    pso = ps_small.tile([128, 512], F32, tag="pss", name="psO")
                    for i in range(SLc // 2):
                        nc.tensor.matmul(
                            pso[:, :sz],
                            eT[:, 2 * i : 2 * i + 2, ts(v, 128)],
                            y_aug[:, 2 * i : 2 * i + 2, off : off + sz],
                            start=(i == 0),
                            stop=(i == SLc // 2 - 1),
                            perf_mode=DR,
                        )
                    # alternate evacuation engines so neither serializes
                    # the PSUM pool rotation
                    if gi == 0:
                        nc.scalar.copy(ot[:, off : off + sz], pso[:, :sz])
                    else:
                        nc.vector.tensor_copy(ot[:, off : off + sz], pso[:, :sz])
                nc.sync.dma_start(outp[b, ts(v, 128), :], ot[:])
            ps4_ctx.__exit__(None, None, None)

    return nc


def _make_core_inputs(x, phi, w1, b1, w2, n_cores=N_CORES):
    B, N, Dd = x.shape
    E, S, _ = phi.shape
    H = w1.shape[2]
    halves = n_cores // E
    SL = S // halves
    Dc, Hc = Dd // 128, H // 128
    bf = ml_dtypes.bfloat16
    f8 = ml_dtypes.float8_e4m3
    ident_f32 = np.eye(128, dtype=np.float32)
    ident_f8 = np.eye(128, dtype=f8)
    xT_full = np.ascontiguousarray(
        x.transpose(0, 2, 1).astype(f8)
    ).reshape(B, Dc, 128, N)
    x_c = np.ascontiguousarray(x.astype(bf))
    in_maps = []
    for c in range(n_cores):
        e, hh = c // halves, c % halves
        phi_loc = phi[e, hh * SL : (hh + 1) * SL, :]
        phiT = np.ascontiguousarray(phi_loc.T.astype(f8)).reshape(Dc, 128, SL)
        in_maps.append(
            {
                "xT": xT_full,
                "xN": x_c,
                "phiT": phiT,
                "w1": np.ascontiguousarray(w1[e].astype(bf)),
                "w2": np.ascontiguousarray((w2[e] * YSCALE).astype(bf)),
                "b1": np.ascontiguousarray(b1[e]).reshape(Hc, 128),
                "identI": ident_f32,
                "identI8": ident_f8,
            }
        )
    return in_maps


def _combine_core_outputs(outs, b2, n_cores=N_CORES):
    E, D = b2.shape
    halves = n_cores // E
    num = np.zeros(outs[0]["outp"][..., :D].shape, dtype=np.float64)
    den = np.zeros(outs[0]["outp"][..., D].shape, dtype=np.float64)
    for c, r in enumerate(outs):
        e = c // halves
        gdl = r["outp"][..., D].astype(np.float64)
        num += r["outp"][..., :D]
        num += gdl[..., None] * b2[e].astype(np.float64)[None, None, :]
        den += gdl
    return (num / den[..., None]).astype(np.float32)


def kernel(x, phi, w1, b1, w2, b2):
    x = np.asarray(x, dtype=np.float32)
    phi = np.asarray(phi, dtype=np.float32)
    w1 = np.asarray(w1, dtype=np.float32)
    b1 = np.asarray(b1, dtype=np.float32)
    w2 = np.asarray(w2, dtype=np.float32)
    b2 = np.asarray(b2, dtype=np.float32)

    B, N, D = x.shape
    E, S, _ = phi.shape
    H = w1.shape[2]
    SL = S // (N_CORES // E)

    nc = bass.Bass(
        "TRN2", target_bir_lowering=False, debug=False, num_devices=N_CORES
    )
    _emit_moe_kernel(nc, B, N, D, SL, H)
    _split_excess_waits(nc)

    in_maps = _make_core_inputs(x, phi, w1, b1, w2)
    res = run_bass_kernel_spmd(nc, in_maps, core_ids=list(range(N_CORES)))
    return _combine_core_outputs(res.results, b2)



# revision 24
# speedup vs baseline: 1.1879x; 1.1879x over previous
"""Trainium2 Bass kernel for nn_MixtureExpertsMlp (MoE soft routing).

Contract: kernel(**inputs) takes the FULL unsharded inputs
(x [4,4096,768], phi [4,1024,768], w1 [4,768,3072], b1 [4,3072],
w2 [4,3072,768], b2 [4,768]) and returns the FULL output [4,4096,768].

Sharding (expert+slot parallel over 8 NeuronCores): core c owns expert
e = c // 2 and slot half h = c % 2, i.e. SL = 512 of that expert's 1024
routing slots. Every core sees all tokens. Per core and per batch b:

  L^T[s, n]    = sum_d phi[s, d] x[b, n, d]        (slots on partitions)
  E^T          = exp(L^T)          (softmax max-subtraction skipped: the
                                    logits are ~N(0,1), well within range)
  ddenom[s]    = sum_n E^T[s, n]                    (via ACT accum_out)
  Et[n, s]     = E^T transposed per 128x128 block via PE matmul against a
                 bf16 identity (no ddenom dependency, so the transposes and
                 slot matmuls pipeline INSIDE phase 1, one tile behind exp)
  slotsU^T[d,s]= sum_n x[b, n, d] Et[n, s]          (unnormalized)
  slots^T[d,s] = slotsU^T[d, s] / ddenom[s]  (normalization folded into the
                 PSUM->SBUF copy: tensor_mul against a broadcast 1/ddenom
                 row, built via PE transpose + 1-partition ones matmul)
  h^T[h', s]   = gelu_tanh(sum_d w1[d, h'] slots^T[d, s] + b1[h'])
  y[s, od]     = sum_h h^T[h, s] w2[h, od]   (s on partitions directly:
                                    lhsT = h^T block, rhs = w2 rows)
  outp[n, :D]  = sum_s E^T[s, n] y[s, :]      (unnormalized combine)
  outp[n, D]   = sum_s E^T[s, n]              (ones column appended to y)

Host-side unshard: the combine softmax normalizer is global over all
E*S slots, so out = (sum_c num_c + sum_c gdl_c * b2[e(c)]) / sum_c gdl_c
where num_c = outp_c[..., :D] and gdl_c = outp_c[..., D]. This also
folds in b2 exactly (per-expert combine mass times b2[e]).

All matmul operands are bf16 (PSUM accumulation stays f32) EXCEPT the
logits and combine matmuls, which run in fp8 e4m3 DoubleRow mode (2x
MAC rate, K=256 per instruction, measured 216ns for a 512-free pair
vs 2x216ns in bf16). fp8 error analysis (numpy-simulated end to end):
quantizing e (the exp values, shifted by SHIFT so the max fits e4m3)
and y (pre-scaled by YSCALE via w2 so values clear the subnormal
floor) perturbs combine/dispatch weights incoherently ACROSS slots,
so the slot-averaged output error stays ~0.7-1%; quantizing x-content
(xn) or w1/w2 perturbs all slots coherently (~2.5-3.5%) and those
matmuls stay bf16. Predicted rel err 1.6e-2 vs the 2e-2 gate.
phi/w1/w2 are SBUF-resident across batches. Emission is
software-pipelined (transpose(v+1) before slots(v), mlp1(h+1) before
mlp2(h)) to keep the PE matmul pipe gap-free (gaps de-ramp the PE
clock 2.4 -> 1.2 GHz); eT is double-buffered so batch b+1's
logits/exp overlap batch b's combine drain.
"""

import numpy as np
import ml_dtypes
from contextlib import ExitStack

import concourse.bass as bass
import concourse.tile as tile
from concourse import mybir
from concourse.bass import ts
from concourse.bass_utils import run_bass_kernel_spmd

F32 = mybir.dt.float32
BF16 = mybir.dt.bfloat16
F8 = mybir.dt.float8e4
DR = mybir.MatmulPerfMode.DoubleRow
AF = mybir.ActivationFunctionType

N_CORES = 8
# softmax shift: exp(logit - SHIFT) keeps the stored fp8 e-values under the
# e4m3 max (240) for the worst logit (~7.2) while keeping the meaningful
# weight mass out of the subnormal range
SHIFT = 2.5
# y is tiny (~0.015 RMS); scale w2 by 16 on the host so the fp8 y_aug values
# sit in e4m3's normal range. The ones (gdl) column is also 16 so the
# num/den ratio on the host is unchanged.
YSCALE = 16.0


# --------------------------------------------------------------------------
# Post-pass: the walrus build in this container enforces the ISA cap of one
# sync-wait per instruction (two for EventSemaphore); Tile's final drain can
# carry more. Hoist excess waits onto fresh same-engine NOPs.
# --------------------------------------------------------------------------
def _split_excess_waits(nc):
    caps = {"InstEventSemaphore": 2}
    n_new = 0
    for f in nc.m.functions:
        for bb in f.blocks:
            i = 0
            insts = bb.instructions
            while i < len(insts):
                ins = insts[i]
                si = ins.sync_info
                cap = caps.get(type(ins).__name__, 1)
                if si is not None and len(si.on_wait) > cap:
                    waits = list(si.on_wait)
                    keep, hoist = waits[-cap:], waits[:-cap]
                    new_nops = []
                    for w in hoist:
                        nop = mybir.InstNoOp(
                            name=nc.get_next_instruction_name(),
                            engine=ins.engine,
                            ins=[],
                            outs=[],
                            sync_info=mybir.SyncInfo(on_wait=[w], on_update=[]),
                        )
                        nc.register_instruction(nop)
                        new_nops.append(nop)
                    ins.sync_info = mybir.SyncInfo(
                        on_wait=keep, on_update=list(si.on_update)
                    )
                    insts[i:i] = new_nops
                    i += len(new_nops)
                    n_new += len(new_nops)
                i += 1
    return n_new


def _bank_splits(off, width, bank=512):
    """Split [off, off+width) at PSUM-bank (512 f32) boundaries."""
    out, cur = [], off
    while cur < off + width:
        nxt = min((cur // bank + 1) * bank, off + width)
        out.append((cur, nxt - cur))
        cur = nxt
    return out


def _emit_moe_kernel(nc, B, N, D, SL, H, act_fn=AF.Gelu_apprx_tanh):
    assert N % 512 == 0 and D % 128 == 0 and SL % 128 == 0 and H % 128 == 0
    Dc, SLc, Hc = D // 128, SL // 128, H // 128
    NT, NV = N // 512, N // 128
    OD = D + 2  # ones column (combine denom) + pad for alignment

    xT = nc.dram_tensor("xT", [B, Dc, 128, N], F8, kind="ExternalInput").ap()
    xN = nc.dram_tensor("xN", [B, N, D], BF16, kind="ExternalInput").ap()
    phiT = nc.dram_tensor("phiT", [Dc, 128, SL], F8, kind="ExternalInput").ap()
    w1 = nc.dram_tensor("w1", [D, H], BF16, kind="ExternalInput").ap()
    w2 = nc.dram_tensor("w2", [H, D], BF16, kind="ExternalInput").ap()
    b1 = nc.dram_tensor("b1", [Hc, 128], F32, kind="ExternalInput").ap()
    identI = nc.dram_tensor("identI", [128, 128], F32, kind="ExternalInput").ap()
    identI8 = nc.dram_tensor("identI8", [128, 128], F8, kind="ExternalInput").ap()
    outp = nc.dram_tensor("outp", [B, N, OD], BF16, kind="ExternalOutput").ap()

    with tile.TileContext(nc) as tc, ExitStack() as ctx:
        pool = lambda name, bufs, space="SBUF": ctx.enter_context(
            tc.tile_pool(name=name, bufs=bufs, space=space)
        )
        singles = pool("singles", 1)
        eT_pool = pool("eT", 2)
        xt_pool = pool("xT", 2)
        xn_pool = pool("xN", 8)
        Dt_pool = pool("Dt", 8)
        slots_pool = pool("slots", 1)
        h_pool = pool("h", 3)
        y_pool = pool("y", 1)
        dd_pool = pool("dd", 2)
        norm_pool = pool("norm", 1)
        out_pool = pool("out", 6)

        # PSUM pools are opened per phase (stack allocator + overlap-deps
        # on release): each phase gets the accumulator banks it needs and
        # the combine phase reuses the freed banks for a deep pso rotation.
        ACC = Dc * 512
        assert ACC == SLc * 768

        # ---- SBUF residents (loaded once, reused across all batches).
        # w1/w2 DMAs (9.4MB) are deferred into batch 0's first tile so they
        # don't queue ahead of the x tiles the first matmuls need.
        phiT_s = singles.tile([128, Dc, SL], F8)
        nc.sync.dma_start(phiT_s[:], phiT.rearrange("k p m -> p k m"))
        w1_s = singles.tile([128, Dc, H], BF16)
        w2_s = singles.tile([128, Hc, D], BF16)
        b1_s = singles.tile([128, Hc], F32)
        nc.sync.dma_start(b1_s[:], b1.rearrange("o p -> p o"))
        ident = singles.tile([128, 128], F32)
        nc.sync.dma_start(ident[:], identI)
        ident_8 = singles.tile([128, 128], F8)
        nc.sync.dma_start(ident_8[:], identI8)
        # Broadcast-matmul weights: the PE rounds the K=1 contraction up to
        # 32 partitions and reads them all, so rows 1-31 must be REAL zeros
        # (garbage there would be accumulated into every output row).
        ones1 = singles.tile([32, 128], BF16)
        nc.vector.memset(ones1[:], 0.0)
        nc.vector.memset(ones1[0:1, :], 1.0)
        ebias = singles.tile([128, 1], F32)
        nc.vector.memset(ebias[:], -SHIFT)

        pref = {}  # batch -> prefetched (xt tile, [xn tiles]) for tile 0
        DEFER = 3  # combine v's per batch deferred into the next batch's
        #            phase 2->3 window (fills the slotsT-evacuation stall)
        pend_comb = [None]

        def emit_combine_v(bb, vv, pool, eT_t, y_t, act_only=False):
            ot = out_pool.tile([128, OD], BF16)
            for gi, (off, sz) in enumerate(((0, 448), (448, OD - 448))):
                pso = pool.tile([128, 512], F32, tag="pss", name="psO")
                for i in range(SLc // 2):
                    nc.tensor.matmul(
                        pso[:, :sz],
                        eT_t[:, 2 * i : 2 * i + 2, ts(vv, 128)],
                        y_t[:, 2 * i : 2 * i + 2, off : off + sz],
                        start=(i == 0),
                        stop=(i == SLc // 2 - 1),
                        perf_mode=DR,
                    )
                # alternate evacuation engines so neither serializes the
                # PSUM pool rotation; in the deferred window DVE is busy
                # with the slotsT muls, so use ACT for both pieces there
                if gi == 0 or act_only:
                    nc.scalar.copy(ot[:, off : off + sz], pso[:, :sz])
                else:
                    nc.vector.tensor_copy(ot[:, off : off + sz], pso[:, :sz])
            nc.sync.dma_start(outp[bb, ts(vv, 128), :], ot[:])

        for b in range(B):
            # ---- phase 1+2 fused: logits/exp, per-block PE transposes, and
            # ---- unnormalized slots accumulation in one dense PE stream ----
            eT = eT_pool.tile([128, SLc, N], F8)
            ddp = dd_pool.tile([128, SLc, NT], F32)
            ps12 = ctx12 = tc.tile_pool(name="ps12", bufs=2, space="PSUM")
            ps12 = ctx12.__enter__()
            psacc_ctx = tc.tile_pool(name="psacc", bufs=1, space="PSUM")
            ps_acc = psacc_ctx.__enter__()
            ps_small = ps12
            accS = ps_acc.tile([128, ACC], F32, tag="acc", name="accS")
            dts, xns = {}, {}

            def emit_T(v):
                psDt = ps_small.tile([128, 512], F32, tag="pss", name="psD")
                for s in range(SLc):
                    nc.tensor.matmul(
                        psDt[:, ts(s, 128)],
                        eT[:, s, ts(v, 128)],
                        ident_8[:],
                        start=True,
                        stop=True,
                    )
                Dt = Dt_pool.tile([128, SL], BF16)
                nc.vector.tensor_copy(Dt[:], psDt[:, :SL])
                dts[v] = Dt

            def emit_S(v):
                Dt, xn = dts.pop(v), xns.pop(v)
                for d in range(Dc):
                    nc.tensor.matmul(
                        accS[:, d * 512 : d * 512 + SL],
                        xn[:, ts(d, 128)],
                        Dt[:],
                        start=(v == 0),
                        stop=(v == NV - 1),
                    )

            # 1/ddenom broadcast-row chain, interleaved into the final tile's
            # T/S pairs so the PE never waits on the DVE/DMA latency:
            #   reduce+recip (DVE, after last exp) -> PE transpose
            #   [128,SLc]->[SLc,128] -> bf16 copy -> flatten DMAs ->
            #   1-partition ones matmul -> broadcast row in SBUF
            rdd = dd_pool.tile([128, SLc], F32, tag="rdd", name="rdd")
            pstS = norm_pool.tile([SLc, 128], BF16, tag="pstS", name="pstS")
            rddF = norm_pool.tile([1, SL], BF16, tag="rddF", name="rddF")
            rddB = norm_pool.tile([128, SL], F32, tag="rddB", name="rddB")

            def norm_chain_a():
                pst = ps_small.tile([128, 512], F32, tag="pss", name="psT")
                nc.tensor.transpose(pst[0:SLc, 0:128], rdd[:], ident[:])
                nc.vector.tensor_copy(pstS[:], pst[0:SLc, 0:128])
                # one DMA per source partition: a single rearranged
                # "p k -> (p k)" DMA under-reports its partition extent to
                # the Tile dependency tracker and races the copy above
                for k in range(SLc):
                    nc.sync.dma_start(rddF[0:1, ts(k, 128)], pstS[k : k + 1, :])

            def norm_chain_b():
                psB = ps_small.tile([128, 512], F32, tag="pss", name="psB")
                nc.tensor.matmul(
                    psB[:, :SL], ones1[0:1, :], rddF[:], start=True, stop=True
                )
                nc.vector.tensor_copy(rddB[:], psB[:, :SL])

            s_queue = []

            def emit_TS_for_tile(t, hooks=()):
                for i, v in enumerate(range(4 * t, 4 * t + 4)):
                    emit_T(v)
                    if s_queue:
                        emit_S(s_queue.pop(0))
                    s_queue.append(v)
                    for hook_i, hook in hooks:
                        if hook_i == i:
                            hook()

            for t in range(NT):
                pxns = None
                xt = xt_pool.tile([128, Dc, 512], F8)
                nc.sync.dma_start(
                    xt[:], xT[b, :, :, ts(t, 512)].rearrange("k p n -> p k n")
                )
                for s in range(SLc):
                    ps = ps_small.tile([128, 512], F32, tag="pss", name="psL")
                    for j in range(Dc // 2):
                        nc.tensor.matmul(
                            ps[:],
                            phiT_s[:, 2 * j : 2 * j + 2, ts(s, 128)],
                            xt[:, 2 * j : 2 * j + 2, :],
                            start=(j == 0),
                            stop=(j == Dc // 2 - 1),
                            perf_mode=DR,
                        )
                    nc.scalar.activation(
                        eT[:, s, ts(t, 512)],
                        ps[:],
                        AF.Exp,
                        bias=ebias[:],
                        accum_out=ddp[:, s, t : t + 1],
                    )
                for v in range(4 * t, 4 * t + 4):
                    if pxns is not None:
                        xns[v] = pxns[v - 4 * t]
                        continue
                    xn = xn_pool.tile([128, D], BF16)
                    nc.sync.dma_start(xn[:], xN[b, ts(v, 128), :])
                    xns[v] = xn
                if b == 0:
                    # stream the 9.4MB of resident weights in per-tile chunks
                    # so they never queue ahead of the x tiles phase 1 needs
                    hchunk = H // NT
                    nc.sync.dma_start(
                        w1_s[:, :, ts(t, hchunk)],
                        w1[:, ts(t, hchunk)].rearrange("(k p) m -> p k m", p=128),
                    )
                    kchunk = Hc // NT
                    nc.sync.dma_start(
                        w2_s[:, ts(t, kchunk), :],
                        w2[ts(t, kchunk * 128), :].rearrange(
                            "(k p) m -> p k m", p=128
                        ),
                    )
                if t == NT - 1:
                    nc.vector.reduce_sum(
                        rdd[:], ddp[:], axis=mybir.AxisListType.X
                    )
                    nc.vector.reciprocal(rdd[:], rdd[:])
                if t >= 1:
                    emit_TS_for_tile(t - 1)
            emit_TS_for_tile(NT - 1, hooks=((0, norm_chain_a), (3, norm_chain_b)))
            while s_queue:
                emit_S(s_queue.pop(0))

            # slots^T = slotsU^T * (1/ddenom), fused into the PSUM evacuation
            slotsT = slots_pool.tile([128, Dc, SL], BF16)
            for d in range(Dc):
                nc.vector.tensor_mul(
                    slotsT[:, d, :], accS[:, d * 512 : d * 512 + SL], rddB[:]
                )
            psacc_ctx.__exit__(None, None, None)
            ctx12.__exit__(None, None, None)
            ps3_ctx = tc.tile_pool(name="ps3", bufs=2, space="PSUM")
            ps_small = ps3_ctx.__enter__()
            psaccy_ctx = tc.tile_pool(name="psaccy", bufs=1, space="PSUM")
            ps_acc = psaccy_ctx.__enter__()

            # ---- phase 3: expert MLP; y accumulated with s on partitions ----
            accY = ps_acc.tile([128, ACC], F32, tag="acc", name="accY")
            # y_aug allocated up front so its gdl/pad memsets and the
            # per-s evacuation copies (interleaved into the final emit_h2
            # below) never gate the first combine matmul
            y_aug = y_pool.tile([128, SLc, OD], F8)
            nc.vector.memset(y_aug[:, :, D : D + 1], YSCALE)
            nc.vector.memset(y_aug[:, :, D + 1 : D + 2], 0.0)

            def emit_h1(h):
                psh = ps_small.tile([128, 512], F32, tag="pss", name="psH")
                for d in range(Dc):
                    nc.tensor.matmul(
                        psh[:, :SL],
                        w1_s[:, d, ts(h, 128)],
                        slotsT[:, d, :],
                        start=(d == 0),
                        stop=(d == Dc - 1),
                    )
                ht = h_pool.tile([128, SL], BF16)
                nc.scalar.activation(
                    ht[:], psh[:, :SL], act_fn, bias=b1_s[:, h : h + 1]
                )
                return ht

            def emit_h2(h, ht, final=False):
                for s in range(SLc):
                    for off, sz in _bank_splits(s * D, D):
                        # start=True clears the has_written bits of the WHOLE
                        # 2KB PSUM bank: only the bank-leading piece may carry
                        # it. A same-bank follower piece relies on that clear:
                        # its first start=False matmul overwrites (bit clear),
                        # later ones accumulate (bit set).
                        bank_first = off % 512 == 0
                        nc.tensor.matmul(
                            accY[:, off : off + sz],
                            ht[:, ts(s, 128)],
                            w2_s[:, h, off - s * D : off - s * D + sz],
                            start=(h == 0 and bank_first),
                            stop=(h == Hc - 1),
                            skip_group_check=not bank_first,
                        )
            pend_h = None
            for h in range(Hc):
                cur = emit_h1(h)
                if pend_h is not None:
                    emit_h2(h - 1, pend_h)
                pend_h = cur
            emit_h2(Hc - 1, pend_h)
            for s in range(SLc):
                srcY = accY[:, s * D : (s + 1) * D]
                h1 = D // 2
                nc.vector.tensor_copy(y_aug[:, s, :h1], srcY[:, :h1])
                nc.scalar.copy(y_aug[:, s, h1:D], srcY[:, h1:])

            psaccy_ctx.__exit__(None, None, None)
            ps3_ctx.__exit__(None, None, None)
            ps4_ctx = tc.tile_pool(name="ps4", bufs=6, space="PSUM")
            ps_small = ps4_ctx.__enter__()
            # ---- phase 4: combine partials + local denominator ----
            # piece split 448/322 balances the ACT/DVE evacuation against the
            # halved (DoubleRow) PE time per v (~1540 cycles)
            for v in range(NV):
                ot = out_pool.tile([128, OD], BF16)
                for gi, (off, sz) in enumerate(((0, 448), (448, OD - 448))):
                    pso = ps_small.tile([128, 512], F32, tag="pss", name="psO")
                    for i in range(SLc // 2):
                        nc.tensor.matmul(
                            pso[:, :sz],
                            eT[:, 2 * i : 2 * i + 2, ts(v, 128)],
                            y_aug[:, 2 * i : 2 * i + 2, off : off + sz],
                            start=(i == 0),
                            stop=(i == SLc // 2 - 1),
                            perf_mode=DR,
                        )
                    # alternate evacuation engines so neither serializes
                    # the PSUM pool rotation
                    if gi == 0:
                        nc.scalar.copy(ot[:, off : off + sz], pso[:, :sz])
                    else:
                        nc.vector.tensor_copy(ot[:, off : off + sz], pso[:, :sz])
                nc.sync.dma_start(outp[b, ts(v, 128), :], ot[:])
            ps4_ctx.__exit__(None, None, None)

    return nc


def _make_core_inputs(x, phi, w1, b1, w2, n_cores=N_CORES):
    B, N, Dd = x.shape
    E, S, _ = phi.shape
    H = w1.shape[2]
    halves = n_cores // E
    SL = S // halves
    Dc, Hc = Dd // 128, H // 128
    bf = ml_dtypes.bfloat16
    f8 = ml_dtypes.float8_e4m3
    ident_f32 = np.eye(128, dtype=np.float32)
    ident_f8 = np.eye(128, dtype=f8)
    xT_full = np.ascontiguousarray(
        x.transpose(0, 2, 1).astype(f8)
    ).reshape(B, Dc, 128, N)
    x_c = np.ascontiguousarray(x.astype(bf))
    in_maps = []
    for c in range(n_cores):
        e, hh = c // halves, c % halves
        phi_loc = phi[e, hh * SL : (hh + 1) * SL, :]
        phiT = np.ascontiguousarray(phi_loc.T.astype(f8)).reshape(Dc, 128, SL)
        in_maps.append(
            {
                "xT": xT_full,
                "xN": x_c,
                "phiT": phiT,
                "w1": np.ascontiguousarray(w1[e].astype(bf)),
                "w2": np.ascontiguousarray((w2[e] * YSCALE).astype(bf)),
                "b1": np.ascontiguousarray(b1[e]).reshape(Hc, 128),
                "identI": ident_f32,
                "identI8": ident_f8,
            }
        )
    return in_maps


def _combine_core_outputs(outs, b2, n_cores=N_CORES):
    E, D = b2.shape
    halves = n_cores // E
    num = np.zeros(outs[0]["outp"][..., :D].shape, dtype=np.float64)
    den = np.zeros(outs[0]["outp"][..., D].shape, dtype=np.float64)
    for c, r in enumerate(outs):
        e = c // halves
        gdl = r["outp"][..., D].astype(np.float64)
        num += r["outp"][..., :D]
        num += gdl[..., None] * b2[e].astype(np.float64)[None, None, :]
        den += gdl
    return (num / den[..., None]).astype(np.float32)


def kernel(x, phi, w1, b1, w2, b2):
    x = np.asarray(x, dtype=np.float32)
    phi = np.asarray(phi, dtype=np.float32)
    w1 = np.asarray(w1, dtype=np.float32)
    b1 = np.asarray(b1, dtype=np.float32)
    w2 = np.asarray(w2, dtype=np.float32)
    b2 = np.asarray(b2, dtype=np.float32)

    B, N, D = x.shape
    E, S, _ = phi.shape
    H = w1.shape[2]
    SL = S // (N_CORES // E)

    nc = bass.Bass(
        "TRN2", target_bir_lowering=False, debug=False, num_devices=N_CORES
    )
    _emit_moe_kernel(nc, B, N, D, SL, H)
    _split_excess_waits(nc)

    in_maps = _make_core_inputs(x, phi, w1, b1, w2)
    res = run_bass_kernel_spmd(nc, in_maps, core_ids=list(range(N_CORES)))
    return _combine_core_outputs(res.results, b2)



# revision 26
# speedup vs baseline: 1.1913x; 1.0029x over previous
"""Trainium2 Bass kernel for nn_MixtureExpertsMlp (MoE soft routing).

Contract: kernel(**inputs) takes the FULL unsharded inputs
(x [4,4096,768], phi [4,1024,768], w1 [4,768,3072], b1 [4,3072],
w2 [4,3072,768], b2 [4,768]) and returns the FULL output [4,4096,768].

Sharding (expert+slot parallel over 8 NeuronCores): core c owns expert
e = c // 2 and slot half h = c % 2, i.e. SL = 512 of that expert's 1024
routing slots. Every core sees all tokens. Per core and per batch b:

  L^T[s, n]    = sum_d phi[s, d] x[b, n, d]        (slots on partitions)
  E^T          = exp(L^T)          (softmax max-subtraction skipped: the
                                    logits are ~N(0,1), well within range)
  ddenom[s]    = sum_n E^T[s, n]                    (via ACT accum_out)
  Et[n, s]     = E^T transposed per 128x128 block via PE matmul against a
                 bf16 identity (no ddenom dependency, so the transposes and
                 slot matmuls pipeline INSIDE phase 1, one tile behind exp)
  slotsU^T[d,s]= sum_n x[b, n, d] Et[n, s]          (unnormalized)
  slots^T[d,s] = slotsU^T[d, s] / ddenom[s]  (normalization folded into the
                 PSUM->SBUF copy: tensor_mul against a broadcast 1/ddenom
                 row, built via PE transpose + 1-partition ones matmul)
  h^T[h', s]   = gelu_tanh(sum_d w1[d, h'] slots^T[d, s] + b1[h'])
  y[s, od]     = sum_h h^T[h, s] w2[h, od]   (s on partitions directly:
                                    lhsT = h^T block, rhs = w2 rows)
  outp[n, :D]  = sum_s E^T[s, n] y[s, :]      (unnormalized combine)
  outp[n, D]   = sum_s E^T[s, n]              (ones column appended to y)

Host-side unshard: the combine softmax normalizer is global over all
E*S slots, so out = (sum_c num_c + sum_c gdl_c * b2[e(c)]) / sum_c gdl_c
where num_c = outp_c[..., :D] and gdl_c = outp_c[..., D]. This also
folds in b2 exactly (per-expert combine mass times b2[e]).

All matmul operands are bf16 (PSUM accumulation stays f32) EXCEPT the
logits and combine matmuls, which run in fp8 e4m3 DoubleRow mode (2x
MAC rate, K=256 per instruction, measured 216ns for a 512-free pair
vs 2x216ns in bf16). fp8 error analysis (numpy-simulated end to end):
quantizing e (the exp values, shifted by SHIFT so the max fits e4m3)
and y (pre-scaled by YSCALE via w2 so values clear the subnormal
floor) perturbs combine/dispatch weights incoherently ACROSS slots,
so the slot-averaged output error stays ~0.7-1%; quantizing x-content
(xn) or w1/w2 perturbs all slots coherently (~2.5-3.5%) and those
matmuls stay bf16. Predicted rel err 1.6e-2 vs the 2e-2 gate.
phi/w1/w2 are SBUF-resident across batches. Emission is
software-pipelined (transpose(v+1) before slots(v), mlp1(h+1) before
mlp2(h)) to keep the PE matmul pipe gap-free (gaps de-ramp the PE
clock 2.4 -> 1.2 GHz); eT is double-buffered so batch b+1's
logits/exp overlap batch b's combine drain.
"""

import numpy as np
import ml_dtypes
from contextlib import ExitStack

import concourse.bass as bass
import concourse.tile as tile
from concourse import mybir
from concourse.bass import ts
from concourse.bass_utils import run_bass_kernel_spmd

F32 = mybir.dt.float32
BF16 = mybir.dt.bfloat16
F8 = mybir.dt.float8e4
DR = mybir.MatmulPerfMode.DoubleRow
AF = mybir.ActivationFunctionType

N_CORES = 8
# softmax shift: exp(logit - SHIFT) keeps the stored fp8 e-values under the
# e4m3 max (240) for the worst logit (~7.2) while keeping the meaningful
# weight mass out of the subnormal range
SHIFT = 2.5
# y is tiny (~0.015 RMS); scale w2 by 16 on the host so the fp8 y_aug values
# sit in e4m3's normal range. The ones (gdl) column is also 16 so the
# num/den ratio on the host is unchanged.
YSCALE = 16.0


# --------------------------------------------------------------------------
# Post-pass: the walrus build in this container enforces the ISA cap of one
# sync-wait per instruction (two for EventSemaphore); Tile's final drain can
# carry more. Hoist excess waits onto fresh same-engine NOPs.
# --------------------------------------------------------------------------
def _split_excess_waits(nc):
    caps = {"InstEventSemaphore": 2}
    n_new = 0
    for f in nc.m.functions:
        for bb in f.blocks:
            i = 0
            insts = bb.instructions
            while i < len(insts):
                ins = insts[i]
                si = ins.sync_info
                cap = caps.get(type(ins).__name__, 1)
                if si is not None and len(si.on_wait) > cap:
                    waits = list(si.on_wait)
                    keep, hoist = waits[-cap:], waits[:-cap]
                    new_nops = []
                    for w in hoist:
                        nop = mybir.InstNoOp(
                            name=nc.get_next_instruction_name(),
                            engine=ins.engine,
                            ins=[],
                            outs=[],
                            sync_info=mybir.SyncInfo(on_wait=[w], on_update=[]),
                        )
                        nc.register_instruction(nop)
                        new_nops.append(nop)
                    ins.sync_info = mybir.SyncInfo(
                        on_wait=keep, on_update=list(si.on_update)
                    )
                    insts[i:i] = new_nops
                    i += len(new_nops)
                    n_new += len(new_nops)
                i += 1
    return n_new


def _bank_splits(off, width, bank=512):
    """Split [off, off+width) at PSUM-bank (512 f32) boundaries."""
    out, cur = [], off
    while cur < off + width:
        nxt = min((cur // bank + 1) * bank, off + width)
        out.append((cur, nxt - cur))
        cur = nxt
    return out


def _emit_moe_kernel(nc, B, N, D, SL, H, act_fn=AF.Gelu_apprx_tanh):
    assert N % 512 == 0 and D % 128 == 0 and SL % 128 == 0 and H % 128 == 0
    Dc, SLc, Hc = D // 128, SL // 128, H // 128
    NT, NV = N // 512, N // 128
    OD = D + 2  # ones column (combine denom) + pad for alignment

    xT = nc.dram_tensor("xT", [B, Dc, 128, N], F8, kind="ExternalInput").ap()
    xN = nc.dram_tensor("xN", [B, N, D], BF16, kind="ExternalInput").ap()
    phiT = nc.dram_tensor("phiT", [Dc, 128, SL], F8, kind="ExternalInput").ap()
    w1 = nc.dram_tensor("w1", [D, H], BF16, kind="ExternalInput").ap()
    w2 = nc.dram_tensor("w2", [H, D], BF16, kind="ExternalInput").ap()
    b1 = nc.dram_tensor("b1", [Hc, 128], F32, kind="ExternalInput").ap()
    identI = nc.dram_tensor("identI", [128, 128], F32, kind="ExternalInput").ap()
    identI8 = nc.dram_tensor("identI8", [128, 128], F8, kind="ExternalInput").ap()
    outp = nc.dram_tensor("outp", [B, N, OD], BF16, kind="ExternalOutput").ap()

    with tile.TileContext(nc) as tc, ExitStack() as ctx:
        pool = lambda name, bufs, space="SBUF": ctx.enter_context(
            tc.tile_pool(name=name, bufs=bufs, space=space)
        )
        singles = pool("singles", 1)
        eT_pool = pool("eT", 2)
        xt_pool = pool("xT", 2)
        xn_pool = pool("xN", 8)
        Dt_pool = pool("Dt", 8)
        slots_pool = pool("slots", 1)
        h_pool = pool("h", 3)
        y_pool = pool("y", 1)
        dd_pool = pool("dd", 2)
        norm_pool = pool("norm", 1)
        out_pool = pool("out", 6)

        # PSUM pools are opened per phase (stack allocator + overlap-deps
        # on release): each phase gets the accumulator banks it needs and
        # the combine phase reuses the freed banks for a deep pso rotation.
        ACC = Dc * 512
        assert ACC == SLc * 768

        # ---- SBUF residents (loaded once, reused across all batches).
        # w1/w2 DMAs (9.4MB) are deferred into batch 0's first tile so they
        # don't queue ahead of the x tiles the first matmuls need.
        phiT_s = singles.tile([128, Dc, SL], F8)
        nc.sync.dma_start(phiT_s[:], phiT.rearrange("k p m -> p k m"))
        w1_s = singles.tile([128, Dc, H], BF16)
        w2_s = singles.tile([128, Hc, D], BF16)
        b1_s = singles.tile([128, Hc], F32)
        nc.sync.dma_start(b1_s[:], b1.rearrange("o p -> p o"))
        ident = singles.tile([128, 128], F32)
        nc.sync.dma_start(ident[:], identI)
        ident_8 = singles.tile([128, 128], F8)
        nc.sync.dma_start(ident_8[:], identI8)
        # Broadcast-matmul weights: the PE rounds the K=1 contraction up to
        # 32 partitions and reads them all, so rows 1-31 must be REAL zeros
        # (garbage there would be accumulated into every output row).
        ones1 = singles.tile([32, 128], BF16)
        nc.vector.memset(ones1[:], 0.0)
        nc.vector.memset(ones1[0:1, :], 1.0)
        ebias = singles.tile([128, 1], F32)
        nc.vector.memset(ebias[:], -SHIFT)

        pref = {}  # batch -> prefetched (xt tile, [xn tiles]) for tile 0
        DEFER = 3  # combine v's per batch deferred into the next batch's
        #            phase 2->3 window (fills the slotsT-evacuation stall)
        pend_comb = [None]

        def emit_combine_v(bb, vv, pool, eT_t, y_t, act_only=False):
            ot = out_pool.tile([128, OD], BF16)
            for gi, (off, sz) in enumerate(((0, 448), (448, OD - 448))):
                pso = pool.tile([128, 512], F32, tag="pss", name="psO")
                for i in range(SLc // 2):
                    nc.tensor.matmul(
                        pso[:, :sz],
                        eT_t[:, 2 * i : 2 * i + 2, ts(vv, 128)],
                        y_t[:, 2 * i : 2 * i + 2, off : off + sz],
                        start=(i == 0),
                        stop=(i == SLc // 2 - 1),
                        perf_mode=DR,
                    )
                # alternate evacuation engines so neither serializes the
                # PSUM pool rotation; in the deferred window DVE is busy
                # with the slotsT muls, so use ACT for both pieces there
                if gi == 0 or act_only:
                    nc.scalar.copy(ot[:, off : off + sz], pso[:, :sz])
                else:
                    nc.vector.tensor_copy(ot[:, off : off + sz], pso[:, :sz])
            nc.sync.dma_start(outp[bb, ts(vv, 128), :], ot[:])

        for b in range(B):
            # ---- phase 1+2 fused: logits/exp, per-block PE transposes, and
            # ---- unnormalized slots accumulation in one dense PE stream ----
            eT = eT_pool.tile([128, SLc, N], F8)
            ddp = dd_pool.tile([128, SLc, NT], F32)
            ps12 = ctx12 = tc.tile_pool(name="ps12", bufs=2, space="PSUM")
            ps12 = ctx12.__enter__()
            psacc_ctx = tc.tile_pool(name="psacc", bufs=1, space="PSUM")
            ps_acc = psacc_ctx.__enter__()
            ps_small = ps12
            accS = ps_acc.tile([128, ACC], F32, tag="acc", name="accS")
            dts, xns = {}, {}

            def emit_T(v):
                psDt = ps_small.tile([128, 512], F32, tag="pss", name="psD")
                for s in range(SLc):
                    nc.tensor.matmul(
                        psDt[:, ts(s, 128)],
                        eT[:, s, ts(v, 128)],
                        ident_8[:],
                        start=True,
                        stop=True,
                    )
                Dt = Dt_pool.tile([128, SL], BF16)
                nc.vector.tensor_copy(Dt[:], psDt[:, :SL])
                dts[v] = Dt

            def emit_S(v):
                Dt, xn = dts.pop(v), xns.pop(v)
                for d in range(Dc):
                    nc.tensor.matmul(
                        accS[:, d * 512 : d * 512 + SL],
                        xn[:, ts(d, 128)],
                        Dt[:],
                        start=(v == 0),
                        stop=(v == NV - 1),
                    )

            # 1/ddenom broadcast-row chain, interleaved into the final tile's
            # T/S pairs so the PE never waits on the DVE/DMA latency:
            #   reduce+recip (DVE, after last exp) -> PE transpose
            #   [128,SLc]->[SLc,128] -> bf16 copy -> flatten DMAs ->
            #   1-partition ones matmul -> broadcast row in SBUF
            rdd = dd_pool.tile([128, SLc], F32, tag="rdd", name="rdd")
            pstS = norm_pool.tile([SLc, 128], BF16, tag="pstS", name="pstS")
            rddF = norm_pool.tile([1, SL], BF16, tag="rddF", name="rddF")
            rddB = norm_pool.tile([128, SL], F32, tag="rddB", name="rddB")

            def norm_chain_a():
                pst = ps_small.tile([128, 512], F32, tag="pss", name="psT")
                nc.tensor.transpose(pst[0:SLc, 0:128], rdd[:], ident[:])
                nc.vector.tensor_copy(pstS[:], pst[0:SLc, 0:128])
                # one DMA per source partition: a single rearranged
                # "p k -> (p k)" DMA under-reports its partition extent to
                # the Tile dependency tracker and races the copy above
                for k in range(SLc):
                    nc.sync.dma_start(rddF[0:1, ts(k, 128)], pstS[k : k + 1, :])

            def norm_chain_b():
                psB = ps_small.tile([128, 512], F32, tag="pss", name="psB")
                nc.tensor.matmul(
                    psB[:, :SL], ones1[0:1, :], rddF[:], start=True, stop=True
                )
                nc.vector.tensor_copy(rddB[:], psB[:, :SL])

            s_queue = []

            def emit_TS_for_tile(t, hooks=()):
                for i, v in enumerate(range(4 * t, 4 * t + 4)):
                    emit_T(v)
                    if s_queue:
                        emit_S(s_queue.pop(0))
                    s_queue.append(v)
                    for hook_i, hook in hooks:
                        if hook_i == i:
                            hook()

            for t in range(NT):
                pxns = None
                xt = xt_pool.tile([128, Dc, 512], F8)
                nc.sync.dma_start(
                    xt[:], xT[b, :, :, ts(t, 512)].rearrange("k p n -> p k n")
                )
                for s in range(SLc):
                    ps = ps_small.tile([128, 512], F32, tag="pss", name="psL")
                    for j in range(Dc // 2):
                        nc.tensor.matmul(
                            ps[:],
                            phiT_s[:, 2 * j : 2 * j + 2, ts(s, 128)],
                            xt[:, 2 * j : 2 * j + 2, :],
                            start=(j == 0),
                            stop=(j == Dc // 2 - 1),
                            perf_mode=DR,
                        )
                    nc.scalar.activation(
                        eT[:, s, ts(t, 512)],
                        ps[:],
                        AF.Exp,
                        bias=ebias[:],
                        accum_out=ddp[:, s, t : t + 1],
                    )
                for v in range(4 * t, 4 * t + 4):
                    if pxns is not None:
                        xns[v] = pxns[v - 4 * t]
                        continue
                    xn = xn_pool.tile([128, D], BF16)
                    nc.sync.dma_start(xn[:], xN[b, ts(v, 128), :])
                    xns[v] = xn
                if b == 0:
                    # stream the 9.4MB of resident weights in per-tile chunks
                    # so they never queue ahead of the x tiles phase 1 needs
                    hchunk = H // NT
                    nc.sync.dma_start(
                        w1_s[:, :, ts(t, hchunk)],
                        w1[:, ts(t, hchunk)].rearrange("(k p) m -> p k m", p=128),
                    )
                    kchunk = Hc // NT
                    nc.sync.dma_start(
                        w2_s[:, ts(t, kchunk), :],
                        w2[ts(t, kchunk * 128), :].rearrange(
                            "(k p) m -> p k m", p=128
                        ),
                    )
                if t == NT - 1:
                    nc.vector.reduce_sum(
                        rdd[:], ddp[:], axis=mybir.AxisListType.X
                    )
                    nc.vector.reciprocal(rdd[:], rdd[:])
                if t >= 1:
                    emit_TS_for_tile(t - 1)
            emit_TS_for_tile(NT - 1, hooks=((0, norm_chain_a), (3, norm_chain_b)))
            while s_queue:
                emit_S(s_queue.pop(0))

            # slots^T = slotsU^T * (1/ddenom), fused into the PSUM evacuation
            slotsT = slots_pool.tile([128, Dc, SL], BF16)
            for d in range(Dc):
                nc.vector.tensor_mul(
                    slotsT[:, d, :], accS[:, d * 512 : d * 512 + SL], rddB[:]
                )
            psacc_ctx.__exit__(None, None, None)
            ctx12.__exit__(None, None, None)
            ps3_ctx = tc.tile_pool(name="ps3", bufs=2, space="PSUM")
            ps_small = ps3_ctx.__enter__()
            psaccy_ctx = tc.tile_pool(name="psaccy", bufs=1, space="PSUM")
            ps_acc = psaccy_ctx.__enter__()

            # ---- phase 3: expert MLP; y accumulated with s on partitions ----
            accY = ps_acc.tile([128, ACC], F32, tag="acc", name="accY")
            # y_aug allocated up front so its gdl/pad memsets and the
            # per-s evacuation copies (interleaved into the final emit_h2
            # below) never gate the first combine matmul
            y_aug = y_pool.tile([128, SLc, OD], F8)
            nc.vector.memset(y_aug[:, :, D : D + 1], YSCALE)
            nc.vector.memset(y_aug[:, :, D + 1 : D + 2], 0.0)

            def emit_h1(h):
                psh = ps_small.tile([128, 512], F32, tag="pss", name="psH")
                for d in range(Dc):
                    nc.tensor.matmul(
                        psh[:, :SL],
                        w1_s[:, d, ts(h, 128)],
                        slotsT[:, d, :],
                        start=(d == 0),
                        stop=(d == Dc - 1),
                    )
                ht = h_pool.tile([128, SL], BF16)
                nc.scalar.activation(
                    ht[:], psh[:, :SL], act_fn, bias=b1_s[:, h : h + 1]
                )
                return ht

            def emit_h2(h, ht, final=False):
                for s in range(SLc):
                    for off, sz in _bank_splits(s * D, D):
                        # start=True clears the has_written bits of the WHOLE
                        # 2KB PSUM bank: only the bank-leading piece may carry
                        # it. A same-bank follower piece relies on that clear:
                        # its first start=False matmul overwrites (bit clear),
                        # later ones accumulate (bit set).
                        bank_first = off % 512 == 0
                        nc.tensor.matmul(
                            accY[:, off : off + sz],
                            ht[:, ts(s, 128)],
                            w2_s[:, h, off - s * D : off - s * D + sz],
                            start=(h == 0 and bank_first),
                            stop=(h == Hc - 1),
                            skip_group_check=not bank_first,
                        )
                    if final:
                        # s's accY region is complete: evacuate to y_aug now
                        # so the copies overlap the remaining s-blocks' matmuls
                        srcY = accY[:, s * D : (s + 1) * D]
                        h1 = D // 2
                        nc.vector.tensor_copy(y_aug[:, s, :h1], srcY[:, :h1])
                        nc.scalar.copy(y_aug[:, s, h1:D], srcY[:, h1:])

            pend_h = None
            for h in range(Hc):
                cur = emit_h1(h)
                if pend_h is not None:
                    emit_h2(h - 1, pend_h)
                pend_h = cur
            emit_h2(Hc - 1, pend_h, final=True)

            psaccy_ctx.__exit__(None, None, None)
            ps3_ctx.__exit__(None, None, None)
            ps4_ctx = tc.tile_pool(name="ps4", bufs=6, space="PSUM")
            ps_small = ps4_ctx.__enter__()
            # ---- phase 4: combine partials + local denominator ----
            # piece split 448/322 balances the ACT/DVE evacuation against the
            # halved (DoubleRow) PE time per v (~1540 cycles)
            for v in range(NV):
                ot = out_pool.tile([128, OD], BF16)
                for gi, (off, sz) in enumerate(((0, 448), (448, OD - 448))):
                    pso = ps_small.tile([128, 512], F32, tag="pss", name="psO")
                    for i in range(SLc // 2):
                        nc.tensor.matmul(
                            pso[:, :sz],
                            eT[:, 2 * i : 2 * i + 2, ts(v, 128)],
                            y_aug[:, 2 * i : 2 * i + 2, off : off + sz],
                            start=(i == 0),
                            stop=(i == SLc // 2 - 1),
                            perf_mode=DR,
                        )
                    # alternate evacuation engines so neither serializes
                    # the PSUM pool rotation
                    if gi == 0:
                        nc.scalar.copy(ot[:, off : off + sz], pso[:, :sz])
                    else:
                        nc.vector.tensor_copy(ot[:, off : off + sz], pso[:, :sz])
                nc.sync.dma_start(outp[b, ts(v, 128), :], ot[:])
            ps4_ctx.__exit__(None, None, None)

    return nc


def _make_core_inputs(x, phi, w1, b1, w2, n_cores=N_CORES):
    B, N, Dd = x.shape
    E, S, _ = phi.shape
    H = w1.shape[2]
    halves = n_cores // E
    SL = S // halves
    Dc, Hc = Dd // 128, H // 128
    bf = ml_dtypes.bfloat16
    f8 = ml_dtypes.float8_e4m3
    ident_f32 = np.eye(128, dtype=np.float32)
    ident_f8 = np.eye(128, dtype=f8)
    xT_full = np.ascontiguousarray(
        x.transpose(0, 2, 1).astype(f8)
    ).reshape(B, Dc, 128, N)
    x_c = np.ascontiguousarray(x.astype(bf))
    in_maps = []
    for c in range(n_cores):
        e, hh = c // halves, c % halves
        phi_loc = phi[e, hh * SL : (hh + 1) * SL, :]
        phiT = np.ascontiguousarray(phi_loc.T.astype(f8)).reshape(Dc, 128, SL)
        in_maps.append(
            {
                "xT": xT_full,
                "xN": x_c,
                "phiT": phiT,
                "w1": np.ascontiguousarray(w1[e].astype(bf)),
                "w2": np.ascontiguousarray((w2[e] * YSCALE).astype(bf)),
                "b1": np.ascontiguousarray(b1[e]).reshape(Hc, 128),
                "identI": ident_f32,
                "identI8": ident_f8,
            }
        )
    return in_maps


def _combine_core_outputs(outs, b2, n_cores=N_CORES):
    E, D = b2.shape
    halves = n_cores // E
    num = np.zeros(outs[0]["outp"][..., :D].shape, dtype=np.float64)
    den = np.zeros(outs[0]["outp"][..., D].shape, dtype=np.float64)
    for c, r in enumerate(outs):
        e = c // halves
        gdl = r["outp"][..., D].astype(np.float64)
        num += r["outp"][..., :D]
        num += gdl[..., None] * b2[e].astype(np.float64)[None, None, :]
        den += gdl
    return (num / den[..., None]).astype(np.float32)


def kernel(x, phi, w1, b1, w2, b2):
    x = np.asarray(x, dtype=np.float32)
    phi = np.asarray(phi, dtype=np.float32)
    w1 = np.asarray(w1, dtype=np.float32)
    b1 = np.asarray(b1, dtype=np.float32)
    w2 = np.asarray(w2, dtype=np.float32)
    b2 = np.asarray(b2, dtype=np.float32)

    B, N, D = x.shape
    E, S, _ = phi.shape
    H = w1.shape[2]
    SL = S // (N_CORES // E)

    nc = bass.Bass(
        "TRN2", target_bir_lowering=False, debug=False, num_devices=N_CORES
    )
    _emit_moe_kernel(nc, B, N, D, SL, H)
    _split_excess_waits(nc)

    in_maps = _make_core_inputs(x, phi, w1, b1, w2)
    res = run_bass_kernel_spmd(nc, in_maps, core_ids=list(range(N_CORES)))
    return _combine_core_outputs(res.results, b2)



# revision 28
# speedup vs baseline: 3.5861x; 3.0103x over previous
"""Trainium2 Bass kernel for nn_MixtureExpertsMlp (MoE soft routing).

Contract: kernel(**inputs) takes the FULL unsharded inputs
(x [4,4096,768], phi [4,1024,768], w1 [4,768,3072], b1 [4,3072],
w2 [4,3072,768], b2 [4,768]) and returns the FULL output [4,4096,768].

Sharding (expert+slot parallel over 8 NeuronCores): core c owns expert
e = c // 2 and slot half h = c % 2, i.e. SL = 512 of that expert's 1024
routing slots. Every core sees all tokens. Per core and per batch b:

  L^T[s, n]    = sum_d phi[s, d] x[b, n, d]        (slots on partitions)
  E^T          = exp(L^T)          (softmax max-subtraction skipped: the
                                    logits are ~N(0,1), well within range)
  ddenom[s]    = sum_n E^T[s, n]                    (via ACT accum_out)
  Et[n, s]     = E^T transposed per 128x128 block via PE matmul against a
                 bf16 identity (no ddenom dependency, so the transposes and
                 slot matmuls pipeline INSIDE phase 1, one tile behind exp)
  slotsU^T[d,s]= sum_n x[b, n, d] Et[n, s]          (unnormalized)
  slots^T[d,s] = slotsU^T[d, s] / ddenom[s]  (normalization folded into the
                 PSUM->SBUF copy: tensor_mul against a broadcast 1/ddenom
                 row, built via PE transpose + 1-partition ones matmul)
  h^T[h', s]   = gelu_tanh(sum_d w1[d, h'] slots^T[d, s] + b1[h'])
  y[s, od]     = sum_h h^T[h, s] w2[h, od]   (s on partitions directly:
                                    lhsT = h^T block, rhs = w2 rows)
  outp[n, :D]  = sum_s E^T[s, n] y[s, :]      (unnormalized combine)
  outp[n, D]   = sum_s E^T[s, n]              (ones column appended to y)

Host-side unshard: the combine softmax normalizer is global over all
E*S slots, so out = (sum_c num_c + sum_c gdl_c * b2[e(c)]) / sum_c gdl_c
where num_c = outp_c[..., :D] and gdl_c = outp_c[..., D]. This also
folds in b2 exactly (per-expert combine mass times b2[e]).

All matmul operands are bf16 (PSUM accumulation stays f32) EXCEPT the
logits and combine matmuls, which run in fp8 e4m3 DoubleRow mode (2x
MAC rate, K=256 per instruction, measured 216ns for a 512-free pair
vs 2x216ns in bf16). fp8 error analysis (numpy-simulated end to end):
quantizing e (the exp values, shifted by SHIFT so the max fits e4m3)
and y (pre-scaled by YSCALE via w2 so values clear the subnormal
floor) perturbs combine/dispatch weights incoherently ACROSS slots,
so the slot-averaged output error stays ~0.7-1%; quantizing x-content
(xn) or w1/w2 perturbs all slots coherently (~2.5-3.5%) and those
matmuls stay bf16. Predicted rel err 1.6e-2 vs the 2e-2 gate.
phi/w1/w2 are SBUF-resident across batches. Emission is
software-pipelined (transpose(v+1) before slots(v), mlp1(h+1) before
mlp2(h)) to keep the PE matmul pipe gap-free (gaps de-ramp the PE
clock 2.4 -> 1.2 GHz); eT is double-buffered so batch b+1's
logits/exp overlap batch b's combine drain.
"""

import numpy as np
import ml_dtypes
from contextlib import ExitStack

import concourse.bass as bass
import concourse.tile as tile
from concourse import mybir
from concourse.bass import ts
from concourse.bass_utils import run_bass_kernel_spmd

F32 = mybir.dt.float32
BF16 = mybir.dt.bfloat16
F8 = mybir.dt.float8e4
DR = mybir.MatmulPerfMode.DoubleRow
AF = mybir.ActivationFunctionType

N_CORES = 8
# softmax shift: exp(logit - SHIFT) keeps the stored fp8 e-values under the
# e4m3 max (240) for the worst logit (~7.2) while keeping the meaningful
# weight mass out of the subnormal range
SHIFT = 2.5
# y is tiny (~0.015 RMS); scale w2 by 16 on the host so the fp8 y_aug values
# sit in e4m3's normal range. The ones (gdl) column is also 16 so the
# num/den ratio on the host is unchanged.
YSCALE = 16.0


# --------------------------------------------------------------------------
# Post-pass: the walrus build in this container enforces the ISA cap of one
# sync-wait per instruction (two for EventSemaphore); Tile's final drain can
# carry more. Hoist excess waits onto fresh same-engine NOPs.
# --------------------------------------------------------------------------
def _split_excess_waits(nc):
    caps = {"InstEventSemaphore": 2}
    n_new = 0
    for f in nc.m.functions:
        for bb in f.blocks:
            i = 0
            insts = bb.instructions
            while i < len(insts):
                ins = insts[i]
                si = ins.sync_info
                cap = caps.get(type(ins).__name__, 1)
                if si is not None and len(si.on_wait) > cap:
                    waits = list(si.on_wait)
                    keep, hoist = waits[-cap:], waits[:-cap]
                    new_nops = []
                    for w in hoist:
                        nop = mybir.InstNoOp(
                            name=nc.get_next_instruction_name(),
                            engine=ins.engine,
                            ins=[],
                            outs=[],
                            sync_info=mybir.SyncInfo(on_wait=[w], on_update=[]),
                        )
                        nc.register_instruction(nop)
                        new_nops.append(nop)
                    ins.sync_info = mybir.SyncInfo(
                        on_wait=keep, on_update=list(si.on_update)
                    )
                    insts[i:i] = new_nops
                    i += len(new_nops)
                    n_new += len(new_nops)
                i += 1
    return n_new


def _bank_splits(off, width, bank=512):
    """Split [off, off+width) at PSUM-bank (512 f32) boundaries."""
    out, cur = [], off
    while cur < off + width:
        nxt = min((cur // bank + 1) * bank, off + width)
        out.append((cur, nxt - cur))
        cur = nxt
    return out


def _emit_moe_kernel(nc, B, N, D, SL, H, act_fn=AF.Gelu_apprx_tanh):
    assert N % 512 == 0 and D % 128 == 0 and SL % 128 == 0 and H % 128 == 0
    Dc, SLc, Hc = D // 128, SL // 128, H // 128
    NT, NV = N // 512, N // 128
    OD = D + 2  # ones column (combine denom) + pad for alignment

    xT = nc.dram_tensor("xT", [B, Dc, 128, N], F8, kind="ExternalInput").ap()
    xN = nc.dram_tensor("xN", [B, N, D], BF16, kind="ExternalInput").ap()
    phiT = nc.dram_tensor("phiT", [Dc, 128, SL], F8, kind="ExternalInput").ap()
    w1 = nc.dram_tensor("w1", [D, H], BF16, kind="ExternalInput").ap()
    w2 = nc.dram_tensor("w2", [H, D], BF16, kind="ExternalInput").ap()
    b1 = nc.dram_tensor("b1", [Hc, 128], F32, kind="ExternalInput").ap()
    identI = nc.dram_tensor("identI", [128, 128], F32, kind="ExternalInput").ap()
    identI8 = nc.dram_tensor("identI8", [128, 128], F8, kind="ExternalInput").ap()
    outp = nc.dram_tensor("outp", [B, N, OD], BF16, kind="ExternalOutput").ap()

    with tile.TileContext(nc) as tc, ExitStack() as ctx:
        pool = lambda name, bufs, space="SBUF": ctx.enter_context(
            tc.tile_pool(name=name, bufs=bufs, space=space)
        )
        singles = pool("singles", 1)
        eT_pool = pool("eT", 2)
        xt_pool = pool("xT", 2)
        xn_pool = pool("xN", 8)
        Dt_pool = pool("Dt", 8)
        slots_pool = pool("slots", 1)
        h_pool = pool("h", 3)
        y_pool = pool("y", 1)
        dd_pool = pool("dd", 2)
        norm_pool = pool("norm", 1)
        out_pool = pool("out", 6)

        # PSUM pools are opened per phase (stack allocator + overlap-deps
        # on release): each phase gets the accumulator banks it needs and
        # the combine phase reuses the freed banks for a deep pso rotation.
        ACC = Dc * 512
        assert ACC == SLc * 768

        # ---- SBUF residents (loaded once, reused across all batches).
        # w1/w2 DMAs (9.4MB) are deferred into batch 0's first tile so they
        # don't queue ahead of the x tiles the first matmuls need.
        phiT_s = singles.tile([128, Dc, SL], F8)
        nc.sync.dma_start(phiT_s[:], phiT.rearrange("k p m -> p k m"))
        w1_s = singles.tile([128, Dc, H], BF16)
        w2_s = singles.tile([128, Hc, D], BF16)
        b1_s = singles.tile([128, Hc], F32)
        nc.sync.dma_start(b1_s[:], b1.rearrange("o p -> p o"))
        ident = singles.tile([128, 128], F32)
        nc.sync.dma_start(ident[:], identI)
        ident_8 = singles.tile([128, 128], F8)
        nc.sync.dma_start(ident_8[:], identI8)
        # Broadcast-matmul weights: the PE rounds the K=1 contraction up to
        # 32 partitions and reads them all, so rows 1-31 must be REAL zeros
        # (garbage there would be accumulated into every output row).
        ones1 = singles.tile([32, 128], BF16)
        nc.vector.memset(ones1[:], 0.0)
        nc.vector.memset(ones1[0:1, :], 1.0)
        ebias = singles.tile([128, 1], F32)
        nc.vector.memset(ebias[:], -SHIFT)

        pref = {}  # batch -> prefetched (xt tile, [xn tiles]) for tile 0
        DEFER = 3  # combine v's per batch deferred into the next batch's
        #            phase 2->3 window (fills the slotsT-evacuation stall)
        pend_comb = [None]

        def emit_combine_v(bb, vv, pool, eT_t, y_t, act_only=False):
            ot = out_pool.tile([128, OD], BF16)
            for gi, (off, sz) in enumerate(((0, 448), (448, OD - 448))):
                pso = pool.tile([128, 512], F32, tag="pss", name="psO")
                for i in range(SLc // 2):
                    nc.tensor.matmul(
                        pso[:, :sz],
                        eT_t[:, 2 * i : 2 * i + 2, ts(vv, 128)],
                        y_t[:, 2 * i : 2 * i + 2, off : off + sz],
                        start=(i == 0),
                        stop=(i == SLc // 2 - 1),
                        perf_mode=DR,
                    )
                # alternate evacuation engines so neither serializes the
                # PSUM pool rotation; in the deferred window DVE is busy
                # with the slotsT muls, so use ACT for both pieces there
                if gi == 0 or act_only:
                    nc.scalar.copy(ot[:, off : off + sz], pso[:, :sz])
                else:
                    nc.vector.tensor_copy(ot[:, off : off + sz], pso[:, :sz])
            nc.sync.dma_start(outp[bb, ts(vv, 128), :], ot[:])

        for b in range(B):
            # ---- phase 1+2 fused: logits/exp, per-block PE transposes, and
            # ---- unnormalized slots accumulation in one dense PE stream ----
            eT = eT_pool.tile([128, SLc, N], F8)
            ddp = dd_pool.tile([128, SLc, NT], F32)
            ps12 = ctx12 = tc.tile_pool(name="ps12", bufs=2, space="PSUM")
            ps12 = ctx12.__enter__()
            psacc_ctx = tc.tile_pool(name="psacc", bufs=1, space="PSUM")
            ps_acc = psacc_ctx.__enter__()
            ps_small = ps12
            accS = ps_acc.tile([128, ACC], F32, tag="acc", name="accS")
            dts, xns = {}, {}

            def emit_T(v):
                psDt = ps_small.tile([128, 512], F32, tag="pss", name="psD")
                for s in range(SLc):
                    nc.tensor.matmul(
                        psDt[:, ts(s, 128)],
                        eT[:, s, ts(v, 128)],
                        ident_8[:],
                        start=True,
                        stop=True,
                    )
                Dt = Dt_pool.tile([128, SL], BF16)
                nc.vector.tensor_copy(Dt[:], psDt[:, :SL])
                dts[v] = Dt

            def emit_S(v):
                Dt, xn = dts.pop(v), xns.pop(v)
                for d in range(Dc):
                    nc.tensor.matmul(
                        accS[:, d * 512 : d * 512 + SL],
                        xn[:, ts(d, 128)],
                        Dt[:],
                        start=(v == 0),
                        stop=(v == NV - 1),
                    )

            # 1/ddenom broadcast-row chain, interleaved into the final tile's
            # T/S pairs so the PE never waits on the DVE/DMA latency:
            #   reduce+recip (DVE, after last exp) -> PE transpose
            #   [128,SLc]->[SLc,128] -> bf16 copy -> flatten DMAs ->
            #   1-partition ones matmul -> broadcast row in SBUF
            rdd = dd_pool.tile([128, SLc], F32, tag="rdd", name="rdd")
            pstS = norm_pool.tile([SLc, 128], BF16, tag="pstS", name="pstS")
            rddF = norm_pool.tile([1, SL], BF16, tag="rddF", name="rddF")
            rddB = norm_pool.tile([128, SL], F32, tag="rddB", name="rddB")

            def norm_chain_a():
                pst = ps_small.tile([128, 512], F32, tag="pss", name="psT")
                nc.tensor.transpose(pst[0:SLc, 0:128], rdd[:], ident[:])
                nc.vector.tensor_copy(pstS[:], pst[0:SLc, 0:128])
                # one DMA per source partition: a single rearranged
                # "p k -> (p k)" DMA under-reports its partition extent to
                # the Tile dependency tracker and races the copy above
                for k in range(SLc):
                    nc.sync.dma_start(rddF[0:1, ts(k, 128)], pstS[k : k + 1, :])

            def norm_chain_b():
                psB = ps_small.tile([128, 512], F32, tag="pss", name="psB")
                nc.tensor.matmul(
                    psB[:, :SL], ones1[0:1, :], rddF[:], start=True, stop=True
                )
                nc.vector.tensor_copy(rddB[:], psB[:, :SL])

            s_queue = []

            def emit_TS_for_tile(t, hooks=()):
                for i, v in enumerate(range(4 * t, 4 * t + 4)):
                    emit_T(v)
                    if s_queue:
                        emit_S(s_queue.pop(0))
                    s_queue.append(v)
                    for hook_i, hook in hooks:
                        if hook_i == i:
                            hook()

            for t in range(NT):
                pxns = None
                xt = xt_pool.tile([128, Dc, 512], F8)
                nc.sync.dma_start(
                    xt[:], xT[b, :, :, ts(t, 512)].rearrange("k p n -> p k n")
                )
                for s in range(SLc):
                    ps = ps_small.tile([128, 512], F32, tag="pss", name="psL")
                    for j in range(Dc // 2):
                        nc.tensor.matmul(
                            ps[:],
                            phiT_s[:, 2 * j : 2 * j + 2, ts(s, 128)],
                            xt[:, 2 * j : 2 * j + 2, :],
                            start=(j == 0),
                            stop=(j == Dc // 2 - 1),
                            perf_mode=DR,
                        )
                    nc.scalar.activation(
                        eT[:, s, ts(t, 512)],
                        ps[:],
                        AF.Exp,
                        bias=ebias[:],
                        accum_out=ddp[:, s, t : t + 1],
                    )
                for v in range(4 * t, 4 * t + 4):
                    if pxns is not None:
                        xns[v] = pxns[v - 4 * t]
                        continue
                    xn = xn_pool.tile([128, D], BF16)
                    nc.sync.dma_start(xn[:], xN[b, ts(v, 128), :])
                    xns[v] = xn
                if b == 0:
                    # stream the 9.4MB of resident weights in per-tile chunks
                    # so they never queue ahead of the x tiles phase 1 needs
                    hchunk = H // NT
                    nc.sync.dma_start(
                        w1_s[:, :, ts(t, hchunk)],
                        w1[:, ts(t, hchunk)].rearrange("(k p) m -> p k m", p=128),
                    )
                    kchunk = Hc // NT
                    nc.sync.dma_start(
                        w2_s[:, ts(t, kchunk), :],
                        w2[ts(t, kchunk * 128), :].rearrange(
                            "(k p) m -> p k m", p=128
                        ),
                    )
                if t == NT - 1:
                    nc.vector.reduce_sum(
                        rdd[:], ddp[:], axis=mybir.AxisListType.X
                    )
                    nc.vector.reciprocal(rdd[:], rdd[:])
                if t >= 1:
                    emit_TS_for_tile(t - 1)
            emit_TS_for_tile(NT - 1, hooks=((0, norm_chain_a), (3, norm_chain_b)))
            while s_queue:
                emit_S(s_queue.pop(0))

            # slots^T = slotsU^T * (1/ddenom), fused into the PSUM evacuation
            slotsT = slots_pool.tile([128, Dc, SL], BF16)
            for d in range(Dc):
                nc.vector.tensor_mul(
                    slotsT[:, d, :], accS[:, d * 512 : d * 512 + SL], rddB[:]
                )
            # previous batch's deferred combine v's: PE work that fills the
            # stall while the DVE drains the slotsT muls above
            if pend_comb[0] is not None:
                pb, peT, py = pend_comb[0]
                pend_comb[0] = None
                for vv in range(NV - DEFER, NV):
                    emit_combine_v(pb, vv, ps_small, peT, py, act_only=True)
            psacc_ctx.__exit__(None, None, None)
            ctx12.__exit__(None, None, None)
            ps3_ctx = tc.tile_pool(name="ps3", bufs=2, space="PSUM")
            ps_small = ps3_ctx.__enter__()
            psaccy_ctx = tc.tile_pool(name="psaccy", bufs=1, space="PSUM")
            ps_acc = psaccy_ctx.__enter__()

            # ---- phase 3: expert MLP; y accumulated with s on partitions ----
            accY = ps_acc.tile([128, ACC], F32, tag="acc", name="accY")
            # y_aug allocated up front so its gdl/pad memsets and the
            # per-s evacuation copies (interleaved into the final emit_h2
            # below) never gate the first combine matmul
            y_aug = y_pool.tile([128, SLc, OD], F8)
            nc.vector.memset(y_aug[:, :, D : D + 1], YSCALE)
            nc.vector.memset(y_aug[:, :, D + 1 : D + 2], 0.0)

            def emit_h1(h):
                psh = ps_small.tile([128, 512], F32, tag="pss", name="psH")
                for d in range(Dc):
                    nc.tensor.matmul(
                        psh[:, :SL],
                        w1_s[:, d, ts(h, 128)],
                        slotsT[:, d, :],
                        start=(d == 0),
                        stop=(d == Dc - 1),
                    )
                ht = h_pool.tile([128, SL], BF16)
                nc.scalar.activation(
                    ht[:], psh[:, :SL], act_fn, bias=b1_s[:, h : h + 1]
                )
                return ht

            def emit_h2(h, ht, final=False):
                for s in range(SLc):
                    for off, sz in _bank_splits(s * D, D):
                        # start=True clears the has_written bits of the WHOLE
                        # 2KB PSUM bank: only the bank-leading piece may carry
                        # it. A same-bank follower piece relies on that clear:
                        # its first start=False matmul overwrites (bit clear),
                        # later ones accumulate (bit set).
                        bank_first = off % 512 == 0
                        nc.tensor.matmul(
                            accY[:, off : off + sz],
                            ht[:, ts(s, 128)],
                            w2_s[:, h, off - s * D : off - s * D + sz],
                            start=(h == 0 and bank_first),
                            stop=(h == Hc - 1),
                            skip_group_check=not bank_first,
                        )
                    if final:
                        # s's accY region is complete: evacuate to y_aug now
                        # so the copies overlap the remaining s-blocks' matmuls
                        srcY = accY[:, s * D : (s + 1) * D]
                        h1 = D // 2
                        nc.vector.tensor_copy(y_aug[:, s, :h1], srcY[:, :h1])
                        nc.scalar.copy(y_aug[:, s, h1:D], srcY[:, h1:])

            pend_h = None
            for h in range(Hc):
                cur = emit_h1(h)
                if pend_h is not None:
                    emit_h2(h - 1, pend_h)
                pend_h = cur
            emit_h2(Hc - 1, pend_h, final=True)

            psaccy_ctx.__exit__(None, None, None)
            ps3_ctx.__exit__(None, None, None)
            ps4_ctx = tc.tile_pool(name="ps4", bufs=6, space="PSUM")
            ps_small = ps4_ctx.__enter__()
            # ---- phase 4: combine partials + local denominator ----
            # piece split 448/322 balances the ACT/DVE evacuation against the
            # halved (DoubleRow) PE time per v (~1540 cycles). The last DEFER
            # v's of every batch except the final one are emitted in the NEXT
            # batch's phase 2->3 window instead.
            n_now = NV - (DEFER if b + 1 < B else 0)
            for v in range(n_now):
                emit_combine_v(b, v, ps_small, eT, y_aug)
            if b + 1 < B:
                pend_comb[0] = (b, eT, y_aug)
            ps4_ctx.__exit__(None, None, None)

    return nc


def _make_core_inputs(x, phi, w1, b1, w2, n_cores=N_CORES):
    B, N, Dd = x.shape
    E, S, _ = phi.shape
    H = w1.shape[2]
    halves = n_cores // E
    SL = S // halves
    Dc, Hc = Dd // 128, H // 128
    bf = ml_dtypes.bfloat16
    f8 = ml_dtypes.float8_e4m3
    ident_f32 = np.eye(128, dtype=np.float32)
    ident_f8 = np.eye(128, dtype=f8)
    xT_full = np.ascontiguousarray(
        x.transpose(0, 2, 1).astype(f8)
    ).reshape(B, Dc, 128, N)
    x_c = np.ascontiguousarray(x.astype(bf))
    in_maps = []
    for c in range(n_cores):
        e, hh = c // halves, c % halves
        phi_loc = phi[e, hh * SL : (hh + 1) * SL, :]
        phiT = np.ascontiguousarray(phi_loc.T.astype(f8)).reshape(Dc, 128, SL)
        in_maps.append(
            {
                "xT": xT_full,
                "xN": x_c,
                "phiT": phiT,
                "w1": np.ascontiguousarray(w1[e].astype(bf)),
                "w2": np.ascontiguousarray((w2[e] * YSCALE).astype(bf)),
                "b1": np.ascontiguousarray(b1[e]).reshape(Hc, 128),
                "identI": ident_f32,
                "identI8": ident_f8,
            }
        )
    return in_maps


def _combine_core_outputs(outs, b2, n_cores=N_CORES):
    E, D = b2.shape
    halves = n_cores // E
    num = np.zeros(outs[0]["outp"][..., :D].shape, dtype=np.float64)
    den = np.zeros(outs[0]["outp"][..., D].shape, dtype=np.float64)
    for c, r in enumerate(outs):
        e = c // halves
        gdl = r["outp"][..., D].astype(np.float64)
        num += r["outp"][..., :D]
        num += gdl[..., None] * b2[e].astype(np.float64)[None, None, :]
        den += gdl
    return (num / den[..., None]).astype(np.float32)


def kernel(x, phi, w1, b1, w2, b2):
    x = np.asarray(x, dtype=np.float32)
    phi = np.asarray(phi, dtype=np.float32)
    w1 = np.asarray(w1, dtype=np.float32)
    b1 = np.asarray(b1, dtype=np.float32)
    w2 = np.asarray(w2, dtype=np.float32)
    b2 = np.asarray(b2, dtype=np.float32)

    B, N, D = x.shape
    E, S, _ = phi.shape
    H = w1.shape[2]
    SL = S // (N_CORES // E)

    nc = bass.Bass(
        "TRN2", target_bir_lowering=False, debug=False, num_devices=N_CORES
    )
    _emit_moe_kernel(nc, B, N, D, SL, H)
    _split_excess_waits(nc)

    in_maps = _make_core_inputs(x, phi, w1, b1, w2)
    res = run_bass_kernel_spmd(nc, in_maps, core_ids=list(range(N_CORES)))
    return _combine_core_outputs(res.results, b2)

